# revision 16
# baseline (speedup 1.0000x reference)
"""Trainium2 Bass kernel for nn_Backflow (gnn_message_passing), v2.

Pure data-parallel: batch B=128 sharded over 8 NeuronCores (16 samples each).

Key structure vs v1 baseline:
- H = x_i*x_j built at DVE 2x_1P rate: xi values pair-duplicated host-side
  into xi2 (xi2[:, 2e+t] = xsT[:, e]) so every operand's innermost AP dim
  is [step 1, count 2]; one 5-dim op per (group, segment). One segment per
  group runs on GpSimd to offload DVE.
- Triangle split into 6 j-split segments of <=512 cols so L2/L3 outputs
  pack multiple column blocks onto PSUM partitions: z2 = [100, 512]
  (5 blocks x 20 rows), z3 = [24, 512] (6 segs x 4 rows). One GELU / one
  Copy ACT per group for z2/z3 instead of per-chunk ops (5x less ACT).
- L2/L3 single accumulated matmuls with block-placed weight copies
  (no per-sample tile_position loop).
- Scatter U into Z tiles: one DMA per (group, segment) using 4-dim APs
  with 2-level partition split on both sides (4 rows -> 2x2 halves).
- Sqrt issued as the first ACT op; with the act-table patch this leaves
  exactly 2 table loads (sqrt set at t=0, gelu set once).
- xs cast f32->bf16 during SWDGE DMA; consts packed into 2 DMAs.
- Nuclear MLP 2-band packed (cols 0:512 at rows 0.., cols 512:1024 at
  rows 64..) to halve its ACT cost.
"""

import sys

sys.path.insert(0, "/opt/trn_rl_repo")

import numpy as np

import concourse.bass as bass
import concourse.tile as tile
from concourse import bacc, mybir

LN2 = 0.6931471805599453
N_CORES = 8
B, N, D, M = 128, 64, 128, 8
BS = B // N_CORES          # samples per core
R = BS * N                 # rows per core = 1024
NCH = R // 128             # 128-row chunks per core = 8
F32 = mybir.dt.float32
BF16 = mybir.dt.bfloat16
AF = mybir.ActivationFunctionType
ALU = mybir.AluOpType

# triangle runs (i-major, j contiguous): (z1_col_off, i0=j0=16r, jl)
# run2 lives in its own hts tile (htsB, z1 cols 2048:2560) so the GpSimd
# H op never shares a tile with the DVE H ops (whole-tile dep tracking
# would serialize the two engines); run3 packs into htsA after run1.
RUNS = [(0, 0, 64), (1024, 16, 48), (2048, 32, 32), (1792, 48, 16)]
PCOLS = 2560
GP_RUN = 2                 # run computed on gpsimd
# L3 sub-matmuls: (run, z2_block, p3_col0, ncols, z2_col0)
SUBS = [
    (0, 0, 0, 512, 0),
    (0, 1, 512, 512, 0),
    (1, 2, 0, 512, 0),
    (1, 3, 512, 256, 0),
    (3, 3, 0, 256, 256),
    (2, 4, 0, 512, 0),
]

# packed const column maps
CB_EYE = 0
CB_WE1 = 128
CB_WN1 = 160         # [128, 64] (51 real cols, zero-padded so matmul
CB_W2 = 224          # writes full 64-partition bands)  5 x [128, 100]
CB_W3 = 724          # 7 x [100(->128), 16] L3 sub-matmul weights
CB_WN2 = 836         # [128, 64]
CB_WN3 = 900         # [128, 64]
CB_OC = 964          # [128, 64]
CB_ZM = 1028         # [128, 1024] diag-halve mask (0.5 diag cols, 1.0 else)
NB = 2052
CF_BE1, CF_BE2, CF_BE3, CF_BN1, CF_BN2, CF_BN3 = 0, 1, 2, 3, 4, 5
CF_CRD = 6
CF_EY4 = 30
NF = 34

_CACHE = {}


def _patch_act_tables():
    """Keep only 2 selectable act-func sets so bacc never reloads mid-run
    beyond the single sqrt->gelu swap."""
    import concourse.bacc as bacc_mod
    from concourse import hw_specs

    if getattr(bacc_mod.get_activation_tables, "_patched", False):
        return
    orig = hw_specs.get_activation_tables
    keep = {"gelu_and_others", "sqrt_and_others"}

    def patched(arch):
        return {k: (v if k in keep else set()) for k, v in orig(arch).items()}

    patched._patched = True
    bacc_mod.get_activation_tables = patched


def build_graph():
    _patch_act_tables()
    nc = bacc.Bacc(
        "TRN2", target_bir_lowering=False, debug=False, num_devices=N_CORES
    )

    def inp(name, shape, dt=F32):
        return nc.dram_tensor(name, shape, dt, kind="ExternalInput").ap()

    rs_d = inp("rs", [R, 3])
    xs_d = inp("xs", [R, D])
    cb_d = inp("CB", [128, NB], BF16)
    cf_d = inp("CF", [128, NF])
    out_d = nc.dram_tensor("out", [R, 3], F32, kind="ExternalOutput").ap()

    with tile.TileContext(nc) as tc:
        _kernel_body(tc, rs_d, xs_d, cb_d, cf_d, out_d)
    nc.compile()
    return nc


def _kernel_body(tc, rs_d, xs_d, cb_d, cf_d, out_d):
    nc = tc.nc
    from contextlib import ExitStack

    ctx = ExitStack()
    with ctx:
        consts = ctx.enter_context(tc.tile_pool(name="consts", bufs=1))
        datap = ctx.enter_context(tc.tile_pool(name="data", bufs=1))
        hpool = ctx.enter_context(tc.tile_pool(name="hp", bufs=2))
        z1pool = ctx.enter_context(tc.tile_pool(name="z1p", bufs=2))
        z3pool = ctx.enter_context(tc.tile_pool(name="z3p", bufs=2))
        zpool = ctx.enter_context(tc.tile_pool(name="zp", bufs=3))
        smallp = ctx.enter_context(tc.tile_pool(name="smallp", bufs=2))
        ps_s = ctx.enter_context(tc.tile_pool(name="ps_s", bufs=2,
                                              space="PSUM"))
        ps_1 = ctx.enter_context(tc.tile_pool(name="ps_1", bufs=2,
                                              space="PSUM"))
        ps_2 = ctx.enter_context(tc.tile_pool(name="ps_2", bufs=2,
                                              space="PSUM"))
        ps_3 = ctx.enter_context(tc.tile_pool(name="ps_3", bufs=1,
                                              space="PSUM"))

        # ---- rs first (unblocks cutoff + rs casts), then packed consts ----
        rs_sb = consts.tile([128, NCH, 3], F32, tag="rs")
        nc.sync.dma_start(
            rs_sb[:], rs_d[:].rearrange("(c p) f -> p c f", p=128))
        cb = consts.tile([128, NB], BF16, tag="cb")
        nc.sync.dma_start(cb[:, 0:CB_W2], cb_d[:, 0:CB_W2])
        nc.sync.dma_start(cb[:, CB_W2:], cb_d[:, CB_W2:])
        cf = consts.tile([128, NF], F32, tag="cf")
        nc.scalar.dma_start(cf[:], cf_d[:])
        eye = cb[:, CB_EYE:CB_EYE + 128]
        we1 = cb[:, CB_WE1:CB_WE1 + 32]
        wn1 = cb[:, CB_WN1:CB_WN1 + 64]
        wn2p = cb[:, CB_WN2:CB_WN2 + 64]
        wn3p = cb[:, CB_WN3:CB_WN3 + 64]
        ocp = cb[:, CB_OC:CB_OC + 64]
        be1 = cf[:, CF_BE1:CF_BE1 + 1]
        be2p = cf[:, CF_BE2:CF_BE2 + 1]
        be3 = cf[:, CF_BE3:CF_BE3 + 1]
        bn1p = cf[:, CF_BN1:CF_BN1 + 1]
        bn2p = cf[:, CF_BN2:CF_BN2 + 1]
        bn3p = cf[:, CF_BN3:CF_BN3 + 1]
        coordsB = cf[:, CF_CRD:CF_CRD + 24]
        ey4p = cf[:, CF_EY4:CF_EY4 + 4]

        rs_bf = consts.tile([128, NCH, 3], BF16, tag="rsbf")
        nc.vector.tensor_copy(rs_bf[:], rs_sb[:])

        # ---- cutoff: d2 -> sqrt (FIRST ACT OP) -> poly -> cut ----
        df = datap.tile([128, NCH, M, 3], F32, tag="df")
        rs_b = rs_sb[:].unsqueeze(2).broadcast_to([128, NCH, M, 3])
        crd = coordsB.rearrange("p (m c) -> p m c", c=3).unsqueeze(1)\
            .broadcast_to([128, NCH, M, 3])
        nc.vector.tensor_sub(df[:], rs_b, crd)
        nc.vector.tensor_mul(df[:], df[:], df[:])
        d2 = datap.tile([128, NCH, M], F32, tag="d2")
        nc.vector.tensor_reduce(d2[:], df[:], mybir.AxisListType.X, ALU.add)
        d2f = d2[:].rearrange("p c m -> p (c m)")
        r1 = datap.tile([128, NCH * M], F32, tag="r1")
        nc.scalar.activation(r1[:], d2f, AF.Sqrt, scale=4.0)
        pa = datap.tile([128, NCH * M], F32, tag="pa")
        nc.vector.tensor_scalar(pa[:], r1[:], 3.0, -8.0, ALU.mult, ALU.add)
        nc.vector.tensor_mul(pa[:], pa[:], r1[:])
        nc.vector.tensor_scalar(pa[:], pa[:], 6.0, None, ALU.add)
        nc.vector.tensor_mul(r1[:], r1[:], r1[:])
        nc.vector.tensor_mul(pa[:], pa[:], r1[:])
        msk = datap.tile([128, NCH * M], mybir.dt.uint8, tag="msk")
        nc.vector.tensor_scalar(msk[:], d2f, 1.0 / 16.0, None, ALU.is_lt)
        cu = datap.tile([128, NCH * M], F32, tag="cu")
        nc.vector.memset(cu[:], 1.0)
        nc.vector.copy_predicated(cu[:], msk[:], pa[:])
        cuv = cu[:].rearrange("p (c m) -> p c m", m=M)
        t1 = datap.tile([128, NCH, 4], F32, tag="t1")
        nc.vector.tensor_mul(t1[:], cuv[:, :, 0:4], cuv[:, :, 4:8])
        t2 = datap.tile([128, NCH, 2], F32, tag="t2")
        nc.vector.tensor_mul(t2[:], t1[:, :, 0:2], t1[:, :, 2:4])
        cut = datap.tile([128, NCH], F32, tag="cut")
        nc.vector.tensor_mul(
            cut[:].unsqueeze(2), t2[:, :, 0:1], t2[:, :, 1:2])

        # ---- xs: cast-DMA f32->bf16 (SWDGE), transpose to xsT, xi2 ----
        xs_bf = datap.tile([128, NCH, 128], BF16, tag="xsb")
        for cc in range(4):
            nc.gpsimd.dma_start(
                xs_bf[:, 2 * cc:2 * cc + 2, :],
                xs_d[256 * cc:256 * (cc + 1), :]
                .rearrange("(a p) d -> p a d", p=128))
        xsT = datap.tile([128, R], BF16, tag="xsT")
        xi2 = datap.tile([128, 2 * R], BF16, tag="xi2")
        for c in range(NCH):
            pT = ps_s.tile([128, 128], BF16, tag="ps", name="pT")
            nc.tensor.transpose(pT[:, 0:128], xs_bf[:, c, :], eye)
            nc.vector.tensor_copy(xsT[:, 128 * c:128 * (c + 1)],
                                  pT[:, 0:128])
            # xi2[:, 2e+t] = xsT[:, e] for this chunk's 2 samples
            nc.scalar.activation(
                xi2[:, 256 * c:256 * (c + 1)]
                .rearrange("p (s e t) -> p s e t", s=2, t=2),
                xsT[:, 128 * c:128 * (c + 1)]
                .rearrange("p (s e) -> p s e", s=2).unsqueeze(3)
                .broadcast_to([128, 2, 64, 2]), AF.Copy)

        # ---- nuclear MLP, 2-band packed (lhsT zero-padded to 64 cols so
        # every band writes its full 64 PSUM partitions) ----
        bf = datap.tile([128, NCH, 3], F32, tag="bf")
        pn1 = ps_1.tile([128, 512], F32, tag="p1", name="pn1")
        nc.tensor.matmul(pn1[0:64, :], wn1, xsT[:, 0:512],
                         tile_position=(0, 0))
        nc.tensor.matmul(pn1[64:128, :], wn1, xsT[:, 512:1024],
                         tile_position=(0, 64))
        g1p = datap.tile([128, 512], BF16, tag="g1p")
        nc.scalar.activation(g1p[:], pn1[:, :], AF.Gelu, bias=bn1p)
        pn2 = ps_2.tile([128, 512], F32, tag="p2", name="pn2")
        nc.tensor.matmul(pn2[0:64, :], wn2p[0:51, :], g1p[0:51, :],
                         tile_position=(0, 0))
        nc.tensor.matmul(pn2[64:128, :], wn2p[64:115, :], g1p[64:115, :],
                         tile_position=(64, 64))
        g2p = datap.tile([128, 512], BF16, tag="g2p")
        nc.scalar.activation(g2p[:], pn2[:, :], AF.Gelu, bias=bn2p)
        pn3 = ps_3.tile([128, 512], F32, tag="p3", name="pn3")
        nc.tensor.matmul(pn3[0:64, :], wn3p[0:20, :], g2p[0:20, :],
                         tile_position=(0, 0))
        nc.tensor.matmul(pn3[64:128, :], wn3p[64:84, :], g2p[64:84, :],
                         tile_position=(64, 64))
        g3p = datap.tile([128, 512], BF16, tag="g3p")
        nc.scalar.activation(g3p[:], pn3[:, :], AF.Identity, bias=bn3p)
        pn4 = ps_1.tile([128, 512], F32, tag="p1", name="pn4")
        nc.tensor.matmul(pn4[0:64, :], ocp[0:8, :], g3p[0:8, :],
                         tile_position=(0, 0))
        nc.tensor.matmul(pn4[64:128, :], ocp[64:72, :], g3p[64:72, :],
                         tile_position=(64, 64))
        sc = datap.tile([128, 512], F32, tag="sc")
        nc.vector.tensor_copy(sc[:], pn4[:, :])
        for c in range(NCH):
            b, cr = c // 4, 128 * (c % 4)
            pT4 = ps_s.tile([128, 4], F32, tag="ps", name="pT4")
            nc.tensor.transpose(pT4[:, 0:4], sc[64 * b:64 * b + 4,
                                               cr:cr + 128],
                                ey4p[64 * b:64 * b + 4, :],
                                tile_position=(64 * b, 0))
            sc4 = smallp.tile([128, 4], F32, tag="sc4", name="sc4", bufs=4)
            nc.vector.tensor_copy(sc4[:], pT4[:, 0:4])
            # bf = rs*sum_m(g) - g@coords via plain TT ops (scalar-AP
            # operands intermittently stall ~3us in the scalar-load path)
            nc.vector.tensor_mul(
                bf[:, c, :], rs_sb[:, c, :],
                sc4[:, 0:1].broadcast_to([128, 3]))
            nc.vector.tensor_sub(bf[:, c, :], bf[:, c, :], sc4[:, 1:4])

        # ---- electron pair MLP, software-pipelined: back-half of
        # group g-1 is issued after the front-half of group g so engine
        # FIFOs never stall the next group's H build ----
        state = {}

        def front(g):
            xig = xi2[:, 512 * g:512 * (g + 1)]\
                .rearrange("p (s x) -> p s x", s=4)
            xjg = xsT[:, 256 * g:256 * (g + 1)]\
                .rearrange("p (s x) -> p s x", s=4)
            hts = hpool.tile([128, 4, 2048], BF16, tag="HA", name="HA")
            htsB = hpool.tile([128, 4, 512], BF16, tag="HB", name="HB")
            for ri, (co, i0, jl) in enumerate(RUNS):
                if ri == GP_RUN:
                    ov4 = htsB[:, :, :]\
                        .rearrange("p s (i j) -> p s i j", i=16)
                    xi4 = xjg[:, :, i0:i0 + 16].unsqueeze(3)\
                        .broadcast_to([128, 4, 16, jl])
                    xj4 = xjg[:, :, i0:i0 + jl].unsqueeze(2)\
                        .broadcast_to([128, 4, 16, jl])
                    nc.gpsimd.tensor_mul(ov4, xi4, xj4)
                    continue
                # DVE per-sample ops; innermost [step 1, count 2] on every
                # operand (xi pair-duplicated) -> 2x_1P mode
                for s in range(4):
                    ov = hts[:, s, co:co + 16 * jl]\
                        .rearrange("p (i jj t) -> p i jj t", i=16, t=2)
                    xi = xig[:, s, 2 * i0:2 * i0 + 32]\
                        .rearrange("p (i t) -> p i t", t=2).unsqueeze(2)\
                        .broadcast_to([128, 16, jl // 2, 2])
                    xj = xjg[:, s, i0:i0 + jl]\
                        .rearrange("p (jj t) -> p jj t", t=2).unsqueeze(1)\
                        .broadcast_to([128, 16, jl // 2, 2])
                    nc.vector.tensor_mul(ov, xi, xj)
            # L1: 5 blocks of 512, 4 sample-band matmuls each
            z1 = z1pool.tile([128, PCOLS], BF16, tag="z1")
            for b in range(5):
                p1 = ps_1.tile([128, 512], F32, tag="p1", name="p1")
                rhs_t = hts[:, :, 512 * b:512 * (b + 1)] if b < 4 else htsB
                for q in range(4):
                    nc.tensor.matmul(
                        p1[32 * q:32 * q + 32, :], we1,
                        rhs_t[:, q, :],
                        tile_position=(0, 32 * q))
                nc.scalar.activation(z1[:, 512 * b:512 * (b + 1)],
                                     p1[:, :], AF.Gelu, bias=be1)
            # L2: accumulate 5 block-placed matmuls into [100, 512]
            p2 = ps_2.tile([128, 512], F32, tag="p2", name="p2")
            for b in range(5):
                nc.tensor.matmul(
                    p2[0:100, :], cb[:, CB_W2 + 100 * b:CB_W2 + 100 * b + 100],
                    z1[:, 512 * b:512 * (b + 1)],
                    start=(b == 0), stop=(b == 4))
            z2 = z1pool.tile([128, 512], BF16, tag="z2", name="z2")
            nc.scalar.activation(z2[0:100, :], p2[0:100, :], AF.Gelu,
                                 bias=be2p[0:100, :])
            # L3: run r at psum rows 4r:4r+4, accumulated over 7 sub-matmuls
            p3 = ps_3.tile([16, 1024], F32, tag="p3", name="p3")
            for ssi, (ri, zb, pc0, ncol, zc0) in enumerate(SUBS):
                w3s = cb[0:100, CB_W3 + 16 * ssi:CB_W3 + 16 * (ssi + 1)]
                nc.tensor.matmul(
                    p3[0:16, pc0:pc0 + ncol], w3s,
                    z2[0:100, zc0:zc0 + ncol],
                    # subs 0/1 are the first writers of cols 0:512/512:1024;
                    # start=True clears stale has_written in their region
                    start=(ssi <= 1), stop=(ssi == len(SUBS) - 1),
                    skip_group_check=True)
            zu = zpool.tile([128, 128], BF16, tag="zu", name="zu")
            nc.vector.memset(zu[:], 0.0)
            state[g] = (p3, zu)

        def mid(g):
            p3, zu = state[g]
            # drain z3 PSUM->SBUF fused with the diagonal-halve mask
            # (0.5 on i==j 16-blocks, which U+U^T counts twice)
            z3g = z3pool.tile([16, 1024], BF16, tag="z3g", name="z3g")
            nc.vector.tensor_mul(z3g[:], p3[0:16, :],
                                 cb[0:16, CB_ZM:CB_ZM + 1024])
            # scatter U rows into Z tile; q = 2*cc + h
            seng = [nc.sync, nc.gpsimd, nc.scalar, nc.sync]
            for ri, (co, i0, jl) in enumerate(RUNS):
                for q in range(4):
                    cc, h = q // 2, q % 2
                    src = z3g[4 * ri + q:4 * ri + q + 1, 0:16 * jl]\
                        .rearrange("p (i j) -> p i j", i=16)
                    dst = zu[64 * h + i0:64 * h + i0 + 16,
                             64 * cc + i0:64 * cc + i0 + jl]
                    seng[q].dma_start(dst, src)

        def back(g):
            p3, zu = state.pop(g)
            # per chunk: Z = U + U^T (+be3), rowsum, Z @ rs, combine
            o2 = smallp.tile([128, 2, 3], F32, tag="oc", name="oc", bufs=2)
            for cc in range(2):
                c = 2 * g + cc
                pU = ps_s.tile([128, 64], BF16, tag="ps", name="pU")
                for h in range(2):
                    pr = slice(64 * h, 64 * (h + 1))
                    nc.tensor.transpose(
                        pU[pr, 0:64], zu[pr, 64 * cc:64 * cc + 64],
                        eye[pr, pr], tile_position=(64 * h, 64 * h))
                zsb = zpool.tile([128, 64], BF16, tag="zf", name="zf",
                                 bufs=4)
                s2 = smallp.tile([128, 1], F32, tag="s2", name="s2", bufs=4)
                nc.vector.scalar_tensor_tensor(
                    zsb[:], zu[:, 64 * cc:64 * cc + 64], be3, pU[:, 0:64],
                    ALU.add, ALU.add, accum_out=s2[:])
                pE = ps_s.tile([128, 3], F32, tag="ps", name="pE")
                for h in range(2):
                    pr = slice(64 * h, 64 * (h + 1))
                    nc.tensor.matmul(pE[pr, 0:3], zsb[pr, :],
                                     rs_bf[pr, c, :],
                                     tile_position=(64 * h, 64 * h))
                tmp = smallp.tile([128, 3], F32, tag="tmpE", name="tmpE",
                                  bufs=4)
                nc.vector.scalar_tensor_tensor(
                    tmp[:], rs_sb[:, c, :], s2[:, 0:1], pE[:, 0:3],
                    ALU.mult, ALU.subtract)
                nc.vector.tensor_add(tmp[:], bf[:, c, :], tmp[:])
                nc.vector.tensor_scalar(o2[:, cc, :], tmp[:],
                                        cut[:, c:c + 1], 1e-4,
                                        ALU.mult, ALU.mult)
                nc.vector.tensor_add(o2[:, cc, :], o2[:, cc, :],
                                     rs_sb[:, c, :])
            # store on sync: a DIRECT2D waiting on o2 in the ACT queue
            # would block the next groups' GELU issue
            nc.sync.dma_start(
                out_d[256 * g:256 * (g + 1), :]
                .rearrange("(cc p) f -> p cc f", p=128), o2[:])

        ng = BS // 4
        for g in range(ng):
            front(g)
            if g >= 1:
                mid(g - 1)
            if g >= 2:
                back(g - 2)
        mid(ng - 1)
        back(ng - 2)
        back(ng - 1)


def prep_inputs(rs, xs, coords, We1, be1, We2, be2, We3, be3,
                Wn1, bn1, Wn2, bn2, Wn3, bn3):
    """Host-side: shard rs/xs over cores, fold -ln2 into biases, pack."""
    import ml_dtypes

    f = np.float32
    bf = ml_dtypes.bfloat16
    rs = np.asarray(rs, f)
    xs = np.asarray(xs, f)
    coords = np.asarray(coords, f)
    We1a, be1a = np.asarray(We1, f), np.asarray(be1, f)
    We2a, be2a = np.asarray(We2, f), np.asarray(be2, f).reshape(5)
    We3a, be3a = np.asarray(We3, f), float(np.asarray(be3, f).reshape(()))
    Wn1a, bn1a = np.asarray(Wn1, f), np.asarray(bn1, f)
    Wn2a, bn2a = np.asarray(Wn2, f), np.asarray(bn2, f)
    Wn3a, bn3a = np.asarray(Wn3, f), np.asarray(bn3, f)

    cbm = np.zeros((128, NB), f)
    cbm[:, CB_EYE:CB_EYE + 128] = np.eye(128, dtype=f)
    cbm[:, CB_WE1:CB_WE1 + 25] = We1a
    cbm[:, CB_WN1:CB_WN1 + 51] = Wn1a
    for b in range(5):
        for q in range(4):
            cbm[32 * q:32 * q + 25,
                CB_W2 + 100 * b + 20 * b + 5 * q:
                CB_W2 + 100 * b + 20 * b + 5 * q + 5] = We2a
    for ssi, (ri, zb, pc0, ncol, zc0) in enumerate(SUBS):
        for q in range(4):
            cbm[20 * zb + 5 * q:20 * zb + 5 * q + 5,
                CB_W3 + 16 * ssi + 4 * ri + q] = We3a[:, 0]
    cbm[0:51, CB_WN2:CB_WN2 + 20] = Wn2a
    cbm[64:115, CB_WN2:CB_WN2 + 20] = Wn2a
    cbm[0:20, CB_WN3:CB_WN3 + 8] = Wn3a
    cbm[64:84, CB_WN3:CB_WN3 + 8] = Wn3a
    oc = np.concatenate([np.ones((8, 1), f), coords], axis=1)
    cbm[0:8, CB_OC:CB_OC + 4] = oc
    cbm[64:72, CB_OC:CB_OC + 4] = oc
    zm = np.ones((128, 1024), f)
    for ri, (co, i0, jl) in enumerate(RUNS):
        v = zm[4 * ri:4 * ri + 4, 0:16 * jl].reshape(4, 16, jl)
        v[:, :, 0:16] = 0.5             # diag block = first 16 j of each run
    cbm[:, CB_ZM:CB_ZM + 1024] = zm

    cfm = np.zeros((128, NF), f)
    for q in range(4):
        cfm[32 * q:32 * q + 25, CF_BE1] = be1a
    for b in range(5):
        for q in range(4):
            cfm[20 * b + 5 * q:20 * b + 5 * q + 5, CF_BE2] = be2a
    cfm[:, CF_BE3] = be3a
    cfm[0:51, CF_BN1] = bn1a
    cfm[64:115, CF_BN1] = bn1a
    cfm[0:20, CF_BN2] = bn2a
    cfm[64:84, CF_BN2] = bn2a
    cfm[0:8, CF_BN3] = bn3a
    cfm[64:72, CF_BN3] = bn3a
    cfm[:, CF_CRD:CF_CRD + 24] = coords.reshape(1, 24)
    cfm[0:4, CF_EY4:CF_EY4 + 4] = np.eye(4, dtype=f)
    cfm[64:68, CF_EY4:CF_EY4 + 4] = np.eye(4, dtype=f)

    shared = dict(
        CB=np.ascontiguousarray(cbm.astype(bf)),
        CF=np.ascontiguousarray(cfm),
    )
    in_maps = []
    for i in range(N_CORES):
        m = dict(shared)
        m["rs"] = np.ascontiguousarray(rs[BS * i:BS * (i + 1)].reshape(R, 3))
        m["xs"] = np.ascontiguousarray(xs[BS * i:BS * (i + 1)].reshape(R, D))
        in_maps.append(m)
    return in_maps


def get_graph():
    if "nc" not in _CACHE:
        _CACHE["nc"] = build_graph()
    return _CACHE["nc"]


def kernel(**inputs):
    from concourse.bass_utils import run_bass_kernel_spmd

    nc = get_graph()
    in_maps = prep_inputs(**inputs)
    res = run_bass_kernel_spmd(nc, in_maps, core_ids=list(range(N_CORES)))
    outs = [res.results[i]["out"].reshape(BS, N, 3) for i in range(N_CORES)]
    return np.concatenate(outs, axis=0)


# revision 18
# speedup vs baseline: 1.0516x; 1.0516x over previous
"""Trainium2 Bass kernel for nn_Backflow (gnn_message_passing), v2.

Pure data-parallel: batch B=128 sharded over 8 NeuronCores (16 samples each).

Key structure vs v1 baseline:
- H = x_i*x_j built at DVE 2x_1P rate: xi values pair-duplicated host-side
  into xi2 (xi2[:, 2e+t] = xsT[:, e]) so every operand's innermost AP dim
  is [step 1, count 2]; one 5-dim op per (group, segment). One segment per
  group runs on GpSimd to offload DVE.
- Triangle split into 6 j-split segments of <=512 cols so L2/L3 outputs
  pack multiple column blocks onto PSUM partitions: z2 = [100, 512]
  (5 blocks x 20 rows), z3 = [24, 512] (6 segs x 4 rows). One GELU / one
  Copy ACT per group for z2/z3 instead of per-chunk ops (5x less ACT).
- L2/L3 single accumulated matmuls with block-placed weight copies
  (no per-sample tile_position loop).
- Scatter U into Z tiles: one DMA per (group, segment) using 4-dim APs
  with 2-level partition split on both sides (4 rows -> 2x2 halves).
- Sqrt issued as the first ACT op; with the act-table patch this leaves
  exactly 2 table loads (sqrt set at t=0, gelu set once).
- xs cast f32->bf16 during SWDGE DMA; consts packed into 2 DMAs.
- Nuclear MLP 2-band packed (cols 0:512 at rows 0.., cols 512:1024 at
  rows 64..) to halve its ACT cost.
"""

import sys

sys.path.insert(0, "/opt/trn_rl_repo")

import numpy as np

import concourse.bass as bass
import concourse.tile as tile
from concourse import bacc, mybir

LN2 = 0.6931471805599453
N_CORES = 8
B, N, D, M = 128, 64, 128, 8
BS = B // N_CORES          # samples per core
R = BS * N                 # rows per core = 1024
NCH = R // 128             # 128-row chunks per core = 8
F32 = mybir.dt.float32
BF16 = mybir.dt.bfloat16
AF = mybir.ActivationFunctionType
ALU = mybir.AluOpType

# triangle runs (i-major, j contiguous): (z1_col_off, i0=j0=16r, jl)
# run2 lives in its own hts tile (htsB, z1 cols 2048:2560) so the GpSimd
# H op never shares a tile with the DVE H ops (whole-tile dep tracking
# would serialize the two engines); run3 packs into htsA after run1.
RUNS = [(0, 0, 64), (1024, 16, 48), (2048, 32, 32), (1792, 48, 16)]
PCOLS = 2560
GP_RUN = 2                 # run computed on gpsimd
# L3 sub-matmuls: (run, z2_block, p3_col0, ncols, z2_col0)
SUBS = [
    (0, 0, 0, 512, 0),
    (0, 1, 512, 512, 0),
    (1, 2, 0, 512, 0),
    (1, 3, 512, 256, 0),
    (3, 3, 0, 256, 256),
    (2, 4, 0, 512, 0),
]

# packed const column maps
CB_EYE = 0
CB_WE1 = 128
CB_WN1 = 160         # [128, 64] (51 real cols, zero-padded so matmul
CB_W2 = 224          # writes full 64-partition bands)  5 x [128, 100]
CB_W3 = 724          # 7 x [100(->128), 16] L3 sub-matmul weights
CB_WN2 = 836         # [128, 64]
CB_WN3 = 900         # [128, 64]
CB_OC = 964          # [128, 64]
CB_ZM = 1028         # [128, 1024] diag-halve mask (0.5 diag cols, 1.0 else)
NB = 2052
CF_BE1, CF_BE2, CF_BE3, CF_BN1, CF_BN2, CF_BN3 = 0, 1, 2, 3, 4, 5
CF_CRD = 6
CF_EY4 = 30
NF = 34

_CACHE = {}


def _patch_act_tables():
    """Keep only 2 selectable act-func sets so bacc never reloads mid-run
    beyond the single sqrt->gelu swap."""
    import concourse.bacc as bacc_mod
    from concourse import hw_specs

    if getattr(bacc_mod.get_activation_tables, "_patched", False):
        return
    orig = hw_specs.get_activation_tables
    keep = {"gelu_and_others", "sqrt_and_others"}

    def patched(arch):
        return {k: (v if k in keep else set()) for k, v in orig(arch).items()}

    patched._patched = True
    bacc_mod.get_activation_tables = patched


def build_graph():
    _patch_act_tables()
    nc = bacc.Bacc(
        "TRN2", target_bir_lowering=False, debug=False, num_devices=N_CORES
    )

    def inp(name, shape, dt=F32):
        return nc.dram_tensor(name, shape, dt, kind="ExternalInput").ap()

    rs_d = inp("rs", [R, 3])
    xs_d = inp("xs", [R, D])
    cb_d = inp("CB", [128, NB], BF16)
    cf_d = inp("CF", [128, NF])
    out_d = nc.dram_tensor("out", [R, 3], F32, kind="ExternalOutput").ap()

    with tile.TileContext(nc) as tc:
        _kernel_body(tc, rs_d, xs_d, cb_d, cf_d, out_d)
    nc.compile()
    return nc


def _kernel_body(tc, rs_d, xs_d, cb_d, cf_d, out_d):
    nc = tc.nc
    from contextlib import ExitStack

    ctx = ExitStack()
    with ctx:
        consts = ctx.enter_context(tc.tile_pool(name="consts", bufs=1))
        datap = ctx.enter_context(tc.tile_pool(name="data", bufs=1))
        hpool = ctx.enter_context(tc.tile_pool(name="hp", bufs=2))
        z1pool = ctx.enter_context(tc.tile_pool(name="z1p", bufs=2))
        z3pool = ctx.enter_context(tc.tile_pool(name="z3p", bufs=2))
        zpool = ctx.enter_context(tc.tile_pool(name="zp", bufs=3))
        smallp = ctx.enter_context(tc.tile_pool(name="smallp", bufs=2))
        ps_s = ctx.enter_context(tc.tile_pool(name="ps_s", bufs=2,
                                              space="PSUM"))
        ps_1 = ctx.enter_context(tc.tile_pool(name="ps_1", bufs=2,
                                              space="PSUM"))
        ps_2 = ctx.enter_context(tc.tile_pool(name="ps_2", bufs=2,
                                              space="PSUM"))
        ps_3 = ctx.enter_context(tc.tile_pool(name="ps_3", bufs=1,
                                              space="PSUM"))

        # ---- rs first (unblocks cutoff + rs casts), then packed consts ----
        rs_sb = consts.tile([128, NCH, 3], F32, tag="rs")
        nc.sync.dma_start(
            rs_sb[:], rs_d[:].rearrange("(c p) f -> p c f", p=128))
        cb = consts.tile([128, NB], BF16, tag="cb")
        nc.sync.dma_start(cb[:, 0:CB_W2], cb_d[:, 0:CB_W2])
        nc.sync.dma_start(cb[:, CB_W2:], cb_d[:, CB_W2:])
        cf = consts.tile([128, NF], F32, tag="cf")
        nc.scalar.dma_start(cf[:], cf_d[:])
        eye = cb[:, CB_EYE:CB_EYE + 128]
        we1 = cb[:, CB_WE1:CB_WE1 + 32]
        wn1 = cb[:, CB_WN1:CB_WN1 + 64]
        wn2p = cb[:, CB_WN2:CB_WN2 + 64]
        wn3p = cb[:, CB_WN3:CB_WN3 + 64]
        ocp = cb[:, CB_OC:CB_OC + 64]
        be1 = cf[:, CF_BE1:CF_BE1 + 1]
        be2p = cf[:, CF_BE2:CF_BE2 + 1]
        be3 = cf[:, CF_BE3:CF_BE3 + 1]
        bn1p = cf[:, CF_BN1:CF_BN1 + 1]
        bn2p = cf[:, CF_BN2:CF_BN2 + 1]
        bn3p = cf[:, CF_BN3:CF_BN3 + 1]
        coordsB = cf[:, CF_CRD:CF_CRD + 24]
        ey4p = cf[:, CF_EY4:CF_EY4 + 4]

        rs_bf = consts.tile([128, NCH, 3], BF16, tag="rsbf")
        nc.vector.tensor_copy(rs_bf[:], rs_sb[:])

        # ---- cutoff: d2 -> sqrt (FIRST ACT OP) -> poly -> cut ----
        df = datap.tile([128, NCH, M, 3], F32, tag="df")
        rs_b = rs_sb[:].unsqueeze(2).broadcast_to([128, NCH, M, 3])
        crd = coordsB.rearrange("p (m c) -> p m c", c=3).unsqueeze(1)\
            .broadcast_to([128, NCH, M, 3])
        nc.vector.tensor_sub(df[:], rs_b, crd)
        nc.vector.tensor_mul(df[:], df[:], df[:])
        d2 = datap.tile([128, NCH, M], F32, tag="d2")
        nc.vector.tensor_reduce(d2[:], df[:], mybir.AxisListType.X, ALU.add)
        d2f = d2[:].rearrange("p c m -> p (c m)")
        r1 = datap.tile([128, NCH * M], F32, tag="r1")
        nc.scalar.activation(r1[:], d2f, AF.Sqrt, scale=4.0)
        pa = datap.tile([128, NCH * M], F32, tag="pa")
        nc.vector.tensor_scalar(pa[:], r1[:], 3.0, -8.0, ALU.mult, ALU.add)
        nc.vector.tensor_mul(pa[:], pa[:], r1[:])
        nc.vector.tensor_scalar(pa[:], pa[:], 6.0, None, ALU.add)
        nc.vector.tensor_mul(r1[:], r1[:], r1[:])
        nc.vector.tensor_mul(pa[:], pa[:], r1[:])
        msk = datap.tile([128, NCH * M], mybir.dt.uint8, tag="msk")
        nc.vector.tensor_scalar(msk[:], d2f, 1.0 / 16.0, None, ALU.is_lt)
        cu = datap.tile([128, NCH * M], F32, tag="cu")
        nc.vector.memset(cu[:], 1.0)
        nc.vector.copy_predicated(cu[:], msk[:], pa[:])
        cuv = cu[:].rearrange("p (c m) -> p c m", m=M)
        t1 = datap.tile([128, NCH, 4], F32, tag="t1")
        nc.vector.tensor_mul(t1[:], cuv[:, :, 0:4], cuv[:, :, 4:8])
        t2 = datap.tile([128, NCH, 2], F32, tag="t2")
        nc.vector.tensor_mul(t2[:], t1[:, :, 0:2], t1[:, :, 2:4])
        cut = datap.tile([128, NCH], F32, tag="cut")
        nc.vector.tensor_mul(
            cut[:].unsqueeze(2), t2[:, :, 0:1], t2[:, :, 1:2])

        # ---- xs: cast-DMA f32->bf16 (SWDGE), transpose to xsT, xi2 ----
        xs_bf = datap.tile([128, NCH, 128], BF16, tag="xsb")
        for cc in range(4):
            nc.gpsimd.dma_start(
                xs_bf[:, 2 * cc:2 * cc + 2, :],
                xs_d[256 * cc:256 * (cc + 1), :]
                .rearrange("(a p) d -> p a d", p=128))
        xsT = datap.tile([128, R], BF16, tag="xsT")
        xi2 = datap.tile([128, 2 * R], BF16, tag="xi2")
        for c in range(NCH):
            pT = ps_s.tile([128, 128], BF16, tag="ps", name="pT")
            nc.tensor.transpose(pT[:, 0:128], xs_bf[:, c, :], eye)
            nc.vector.tensor_copy(xsT[:, 128 * c:128 * (c + 1)],
                                  pT[:, 0:128])
            # xi2[:, 2e+t] = xsT[:, e] for this chunk's 2 samples
            nc.scalar.activation(
                xi2[:, 256 * c:256 * (c + 1)]
                .rearrange("p (s e t) -> p s e t", s=2, t=2),
                xsT[:, 128 * c:128 * (c + 1)]
                .rearrange("p (s e) -> p s e", s=2).unsqueeze(3)
                .broadcast_to([128, 2, 64, 2]), AF.Copy)

        # ---- nuclear MLP, 2-band packed (lhsT zero-padded to 64 cols so
        # every band writes its full 64 PSUM partitions) ----
        bf = datap.tile([128, NCH, 3], F32, tag="bf")
        pn1 = ps_1.tile([128, 512], F32, tag="p1", name="pn1")
        nc.tensor.matmul(pn1[0:64, :], wn1, xsT[:, 0:512],
                         tile_position=(0, 0))
        nc.tensor.matmul(pn1[64:128, :], wn1, xsT[:, 512:1024],
                         tile_position=(0, 64))
        g1p = datap.tile([128, 512], BF16, tag="g1p")
        nc.scalar.activation(g1p[:], pn1[:, :], AF.Gelu, bias=bn1p)
        pn2 = ps_2.tile([128, 512], F32, tag="p2", name="pn2")
        nc.tensor.matmul(pn2[0:64, :], wn2p[0:51, :], g1p[0:51, :],
                         tile_position=(0, 0))
        nc.tensor.matmul(pn2[64:128, :], wn2p[64:115, :], g1p[64:115, :],
                         tile_position=(64, 64))
        g2p = datap.tile([128, 512], BF16, tag="g2p")
        nc.scalar.activation(g2p[:], pn2[:, :], AF.Gelu, bias=bn2p)
        pn3 = ps_3.tile([128, 512], F32, tag="p3", name="pn3")
        nc.tensor.matmul(pn3[0:64, :], wn3p[0:20, :], g2p[0:20, :],
                         tile_position=(0, 0))
        nc.tensor.matmul(pn3[64:128, :], wn3p[64:84, :], g2p[64:84, :],
                         tile_position=(64, 64))
        g3p = datap.tile([128, 512], BF16, tag="g3p")
        nc.scalar.activation(g3p[:], pn3[:, :], AF.Identity, bias=bn3p)
        pn4 = ps_1.tile([128, 512], F32, tag="p1", name="pn4")
        nc.tensor.matmul(pn4[0:64, :], ocp[0:8, :], g3p[0:8, :],
                         tile_position=(0, 0))
        nc.tensor.matmul(pn4[64:128, :], ocp[64:72, :], g3p[64:72, :],
                         tile_position=(64, 64))
        sc = datap.tile([128, 512], F32, tag="sc")
        nc.vector.tensor_copy(sc[:], pn4[:, :])
        for c in range(NCH):
            b, cr = c // 4, 128 * (c % 4)
            pT4 = ps_s.tile([128, 4], F32, tag="ps", name="pT4")
            nc.tensor.transpose(pT4[:, 0:4], sc[64 * b:64 * b + 4,
                                               cr:cr + 128],
                                ey4p[64 * b:64 * b + 4, :],
                                tile_position=(64 * b, 0))
            sc4 = smallp.tile([128, 4], F32, tag="sc4", name="sc4", bufs=4)
            nc.vector.tensor_copy(sc4[:], pT4[:, 0:4])
            # bf = rs*sum_m(g) - g@coords via plain TT ops (scalar-AP
            # operands intermittently stall ~3us in the scalar-load path)
            nc.vector.tensor_mul(
                bf[:, c, :], rs_sb[:, c, :],
                sc4[:, 0:1].broadcast_to([128, 3]))
            nc.vector.tensor_sub(bf[:, c, :], bf[:, c, :], sc4[:, 1:4])

        # ---- electron pair MLP, software-pipelined: back-half of
        # group g-1 is issued after the front-half of group g so engine
        # FIFOs never stall the next group's H build ----
        state = {}

        def front(g):
            xig = xi2[:, 512 * g:512 * (g + 1)]\
                .rearrange("p (s x) -> p s x", s=4)
            xjg = xsT[:, 256 * g:256 * (g + 1)]\
                .rearrange("p (s x) -> p s x", s=4)
            hts = hpool.tile([128, 4, 2048], BF16, tag="HA", name="HA")
            htsB = hpool.tile([128, 4, 512], BF16, tag="HB", name="HB")
            for ri, (co, i0, jl) in enumerate(RUNS):
                if ri == GP_RUN:
                    ov4 = htsB[:, :, :]\
                        .rearrange("p s (i j) -> p s i j", i=16)
                    xi4 = xjg[:, :, i0:i0 + 16].unsqueeze(3)\
                        .broadcast_to([128, 4, 16, jl])
                    xj4 = xjg[:, :, i0:i0 + jl].unsqueeze(2)\
                        .broadcast_to([128, 4, 16, jl])
                    nc.gpsimd.tensor_mul(ov4, xi4, xj4)
                    continue
                # DVE per-sample ops; innermost [step 1, count 2] on every
                # operand (xi pair-duplicated) -> 2x_1P mode
                for s in range(4):
                    ov = hts[:, s, co:co + 16 * jl]\
                        .rearrange("p (i jj t) -> p i jj t", i=16, t=2)
                    xi = xig[:, s, 2 * i0:2 * i0 + 32]\
                        .rearrange("p (i t) -> p i t", t=2).unsqueeze(2)\
                        .broadcast_to([128, 16, jl // 2, 2])
                    xj = xjg[:, s, i0:i0 + jl]\
                        .rearrange("p (jj t) -> p jj t", t=2).unsqueeze(1)\
                        .broadcast_to([128, 16, jl // 2, 2])
                    nc.vector.tensor_mul(ov, xi, xj)
            # L1: 5 blocks of 512, 4 sample-band matmuls each
            z1 = z1pool.tile([128, PCOLS], BF16, tag="z1")
            for b in range(5):
                p1 = ps_1.tile([128, 512], F32, tag="p1", name="p1")
                rhs_t = hts[:, :, 512 * b:512 * (b + 1)] if b < 4 else htsB
                for q in range(4):
                    nc.tensor.matmul(
                        p1[32 * q:32 * q + 32, :], we1,
                        rhs_t[:, q, :],
                        tile_position=(0, 32 * q))
                nc.scalar.activation(z1[:, 512 * b:512 * (b + 1)],
                                     p1[:, :], AF.Gelu, bias=be1)
            # L2: accumulate 5 block-placed matmuls into [100, 512]
            p2 = ps_2.tile([128, 512], F32, tag="p2", name="p2")
            for b in range(5):
                nc.tensor.matmul(
                    p2[0:100, :], cb[:, CB_W2 + 100 * b:CB_W2 + 100 * b + 100],
                    z1[:, 512 * b:512 * (b + 1)],
                    start=(b == 0), stop=(b == 4))
            z2 = z1pool.tile([128, 512], BF16, tag="z2", name="z2")
            nc.scalar.activation(z2[0:100, :], p2[0:100, :], AF.Gelu,
                                 bias=be2p[0:100, :])
            # L3: run r at psum rows 4r:4r+4, accumulated over 7 sub-matmuls
            p3 = ps_3.tile([16, 1024], F32, tag="p3", name="p3")
            for ssi, (ri, zb, pc0, ncol, zc0) in enumerate(SUBS):
                w3s = cb[0:100, CB_W3 + 16 * ssi:CB_W3 + 16 * (ssi + 1)]
                nc.tensor.matmul(
                    p3[0:16, pc0:pc0 + ncol], w3s,
                    z2[0:100, zc0:zc0 + ncol],
                    # subs 0/1 are the first writers of cols 0:512/512:1024;
                    # start=True clears stale has_written in their region
                    start=(ssi <= 1), stop=(ssi == len(SUBS) - 1),
                    skip_group_check=True)
            # zu lives front(g)..back(g) across the 2-stage lag; 4 bufs so
            # front(g+2)'s memset never waits on back(g)'s Z assembly
            zu = zpool.tile([128, 128], BF16, tag="zu", name="zu", bufs=4)
            nc.vector.memset(zu[:], 0.0)
            state[g] = (p3, zu)

        def mid(g):
            p3, zu = state[g]
            # drain z3 PSUM->SBUF fused with the diagonal-halve mask
            # (0.5 on i==j 16-blocks, which U+U^T counts twice)
            z3g = z3pool.tile([16, 1024], BF16, tag="z3g", name="z3g")
            nc.vector.tensor_mul(z3g[:], p3[0:16, :],
                                 cb[0:16, CB_ZM:CB_ZM + 1024])
            # scatter U rows into Z tile; q = 2*cc + h
            seng = [nc.sync, nc.gpsimd, nc.scalar, nc.sync]
            for ri, (co, i0, jl) in enumerate(RUNS):
                for q in range(4):
                    cc, h = q // 2, q % 2
                    src = z3g[4 * ri + q:4 * ri + q + 1, 0:16 * jl]\
                        .rearrange("p (i j) -> p i j", i=16)
                    dst = zu[64 * h + i0:64 * h + i0 + 16,
                             64 * cc + i0:64 * cc + i0 + jl]
                    seng[q].dma_start(dst, src)

        def back(g):
            p3, zu = state.pop(g)
            # per chunk: Z = U + U^T (+be3), rowsum, Z @ rs, combine
            o2 = smallp.tile([128, 2, 3], F32, tag="oc", name="oc", bufs=2)
            for cc in range(2):
                c = 2 * g + cc
                pU = ps_s.tile([128, 64], BF16, tag="ps", name="pU")
                for h in range(2):
                    pr = slice(64 * h, 64 * (h + 1))
                    nc.tensor.transpose(
                        pU[pr, 0:64], zu[pr, 64 * cc:64 * cc + 64],
                        eye[pr, pr], tile_position=(64 * h, 64 * h))
                zsb = zpool.tile([128, 64], BF16, tag="zf", name="zf",
                                 bufs=4)
                s2 = smallp.tile([128, 1], F32, tag="s2", name="s2", bufs=4)
                nc.vector.scalar_tensor_tensor(
                    zsb[:], zu[:, 64 * cc:64 * cc + 64], be3, pU[:, 0:64],
                    ALU.add, ALU.add, accum_out=s2[:])
                pE = ps_s.tile([128, 3], F32, tag="ps", name="pE")
                for h in range(2):
                    pr = slice(64 * h, 64 * (h + 1))
                    nc.tensor.matmul(pE[pr, 0:3], zsb[pr, :],
                                     rs_bf[pr, c, :],
                                     tile_position=(64 * h, 64 * h))
                tmp = smallp.tile([128, 3], F32, tag="tmpE", name="tmpE",
                                  bufs=4)
                nc.vector.scalar_tensor_tensor(
                    tmp[:], rs_sb[:, c, :], s2[:, 0:1], pE[:, 0:3],
                    ALU.mult, ALU.subtract)
                nc.vector.tensor_add(tmp[:], bf[:, c, :], tmp[:])
                nc.vector.tensor_scalar(o2[:, cc, :], tmp[:],
                                        cut[:, c:c + 1], 1e-4,
                                        ALU.mult, ALU.mult)
                nc.vector.tensor_add(o2[:, cc, :], o2[:, cc, :],
                                     rs_sb[:, c, :])
            nc.scalar.dma_start(
                out_d[256 * g:256 * (g + 1), :]
                .rearrange("(cc p) f -> p cc f", p=128), o2[:])

        ng = BS // 4
        for g in range(ng):
            front(g)
            if g >= 1:
                mid(g - 1)
            if g >= 2:
                back(g - 2)
        mid(ng - 1)
        back(ng - 2)
        back(ng - 1)


def prep_inputs(rs, xs, coords, We1, be1, We2, be2, We3, be3,
                Wn1, bn1, Wn2, bn2, Wn3, bn3):
    """Host-side: shard rs/xs over cores, fold -ln2 into biases, pack."""
    import ml_dtypes

    f = np.float32
    bf = ml_dtypes.bfloat16
    rs = np.asarray(rs, f)
    xs = np.asarray(xs, f)
    coords = np.asarray(coords, f)
    We1a, be1a = np.asarray(We1, f), np.asarray(be1, f)
    We2a, be2a = np.asarray(We2, f), np.asarray(be2, f).reshape(5)
    We3a, be3a = np.asarray(We3, f), float(np.asarray(be3, f).reshape(()))
    Wn1a, bn1a = np.asarray(Wn1, f), np.asarray(bn1, f)
    Wn2a, bn2a = np.asarray(Wn2, f), np.asarray(bn2, f)
    Wn3a, bn3a = np.asarray(Wn3, f), np.asarray(bn3, f)

    cbm = np.zeros((128, NB), f)
    cbm[:, CB_EYE:CB_EYE + 128] = np.eye(128, dtype=f)
    cbm[:, CB_WE1:CB_WE1 + 25] = We1a
    cbm[:, CB_WN1:CB_WN1 + 51] = Wn1a
    for b in range(5):
        for q in range(4):
            cbm[32 * q:32 * q + 25,
                CB_W2 + 100 * b + 20 * b + 5 * q:
                CB_W2 + 100 * b + 20 * b + 5 * q + 5] = We2a
    for ssi, (ri, zb, pc0, ncol, zc0) in enumerate(SUBS):
        for q in range(4):
            cbm[20 * zb + 5 * q:20 * zb + 5 * q + 5,
                CB_W3 + 16 * ssi + 4 * ri + q] = We3a[:, 0]
    cbm[0:51, CB_WN2:CB_WN2 + 20] = Wn2a
    cbm[64:115, CB_WN2:CB_WN2 + 20] = Wn2a
    cbm[0:20, CB_WN3:CB_WN3 + 8] = Wn3a
    cbm[64:84, CB_WN3:CB_WN3 + 8] = Wn3a
    oc = np.concatenate([np.ones((8, 1), f), coords], axis=1)
    cbm[0:8, CB_OC:CB_OC + 4] = oc
    cbm[64:72, CB_OC:CB_OC + 4] = oc
    zm = np.ones((128, 1024), f)
    for ri, (co, i0, jl) in enumerate(RUNS):
        v = zm[4 * ri:4 * ri + 4, 0:16 * jl].reshape(4, 16, jl)
        v[:, :, 0:16] = 0.5             # diag block = first 16 j of each run
    cbm[:, CB_ZM:CB_ZM + 1024] = zm

    cfm = np.zeros((128, NF), f)
    for q in range(4):
        cfm[32 * q:32 * q + 25, CF_BE1] = be1a
    for b in range(5):
        for q in range(4):
            cfm[20 * b + 5 * q:20 * b + 5 * q + 5, CF_BE2] = be2a
    cfm[:, CF_BE3] = be3a
    cfm[0:51, CF_BN1] = bn1a
    cfm[64:115, CF_BN1] = bn1a
    cfm[0:20, CF_BN2] = bn2a
    cfm[64:84, CF_BN2] = bn2a
    cfm[0:8, CF_BN3] = bn3a
    cfm[64:72, CF_BN3] = bn3a
    cfm[:, CF_CRD:CF_CRD + 24] = coords.reshape(1, 24)
    cfm[0:4, CF_EY4:CF_EY4 + 4] = np.eye(4, dtype=f)
    cfm[64:68, CF_EY4:CF_EY4 + 4] = np.eye(4, dtype=f)

    shared = dict(
        CB=np.ascontiguousarray(cbm.astype(bf)),
        CF=np.ascontiguousarray(cfm),
    )
    in_maps = []
    for i in range(N_CORES):
        m = dict(shared)
        m["rs"] = np.ascontiguousarray(rs[BS * i:BS * (i + 1)].reshape(R, 3))
        m["xs"] = np.ascontiguousarray(xs[BS * i:BS * (i + 1)].reshape(R, D))
        in_maps.append(m)
    return in_maps


def get_graph():
    if "nc" not in _CACHE:
        _CACHE["nc"] = build_graph()
    return _CACHE["nc"]


def kernel(**inputs):
    from concourse.bass_utils import run_bass_kernel_spmd

    nc = get_graph()
    in_maps = prep_inputs(**inputs)
    res = run_bass_kernel_spmd(nc, in_maps, core_ids=list(range(N_CORES)))
    outs = [res.results[i]["out"].reshape(BS, N, 3) for i in range(N_CORES)]
    return np.concatenate(outs, axis=0)
